# revision 1
# baseline (speedup 1.0000x reference)
"""Jones congruence kernel (V_p = J1 @ V_m @ J2^T per baseline/time/freq) on 8 trn2 cores.

Sharding: time axis (64) split across 8 cores (8 t-steps each). The antenna
gather indices (ant1/ant2) are identical on every core, so the gather offsets
are baked into the single SPMD program as compile-time SBUF addresses.
Interior compute in fp16 (DVE tensor_tensor 2x mode); HBM I/O stays f32 with
SWDGE cast-during-DMA.
"""
import sys
sys.path.insert(0, "/opt/trn_rl_repo")
import numpy as np

NPOL, NANT, NBL, NTIMES, NFREQS = 2, 64, 2016, 64, 256
N_CORES = 8
T_LOC = NTIMES // N_CORES          # 8 timesteps per core
PLANE = T_LOC * NFREQS             # 2048 elems per (ab, baseline) plane
GROUP = 126                        # baselines per tile group
N_GROUPS = NBL // GROUP            # 16

_cache = {}


def _split_excess_waits(nc, mybir):
    """Walrus in this env rejects >2 sem-wait conditions per instruction.
    Insert Drain clones carrying the excess waits immediately before."""
    import copy
    fn = nc.m.functions[0]

    def walk(blocks):
        for bb in blocks:
            yield bb
            yield from walk(getattr(bb, "blocks", None) or [])

    ctr = [0]
    for bb in walk(fn.blocks):
        newlist = []
        for ins in bb.instructions:
            si = ins.sync_info
            if si is not None and si.on_wait and len(si.on_wait) > 1:
                waits = list(si.on_wait)
                while len(waits) > 1:
                    chunk, waits = waits[:1], waits[1:]
                    d = mybir.InstNoOp(
                        name=f"waitsplit-{ctr[0]}",
                        engine=ins.engine,
                        ins=[],
                        outs=[],
                        sync_info=mybir.SyncInfo(on_wait=chunk, on_update=[]),
                    )
                    ctr[0] += 1
                    newlist.append(d)
                si.on_wait = waits
            newlist.append(ins)
        bb.instructions = newlist


def _build(ant1, ant2):
    import concourse.bass as bass
    import concourse.tile as tile
    from concourse import mybir
    from contextlib import ExitStack

    f32, f16 = mybir.dt.float32, mybir.dt.float16
    nc = bass.Bass("TRN2", target_bir_lowering=False, debug=False)
    V = nc.dram_tensor("V", [4, NBL, PLANE], f32, kind="ExternalInput").ap()
    J = nc.dram_tensor("J", [4, NANT, PLANE], f32, kind="ExternalInput").ap()
    O = nc.dram_tensor("O", [4, NBL, PLANE], f32, kind="ExternalOutput").ap()

    with tile.TileContext(nc) as tc:
        with ExitStack() as ctx:
            tabp = ctx.enter_context(tc.tile_pool(name="tab", bufs=1))
            iop = ctx.enter_context(tc.tile_pool(name="io", bufs=2))
            tmpp = ctx.enter_context(tc.tile_pool(name="tmp", bufs=2))

            # jones table: [64 ant partitions, 4 ab planes x 2048] fp16
            tab = tabp.tile([NANT, 4 * PLANE], f16)
            for q in range(4):
                nc.gpsimd.dma_start(tab[:, q * PLANE:(q + 1) * PLANE], J[q])

            for g in range(N_GROUPS):
                n0 = g * GROUP
                vt = iop.tile([GROUP, 4 * PLANE], f16, tag="vt")
                for q in range(4):
                    nc.gpsimd.dma_start(vt[:, q * PLANE:(q + 1) * PLANE],
                                        V[q, n0:n0 + GROUP, :])
                j1 = iop.tile([GROUP, 4 * PLANE], f16, tag="j1")
                j2 = iop.tile([GROUP, 4 * PLANE], f16, tag="j2")
                for p in range(GROUP):
                    e1 = nc.sync if p % 2 == 0 else nc.scalar
                    e2 = nc.scalar if p % 2 == 0 else nc.sync
                    e1.dma_start(j1[p:p + 1, :], tab[ant1[n0 + p]:ant1[n0 + p] + 1, :])
                    e2.dma_start(j2[p:p + 1, :], tab[ant2[n0 + p]:ant2[n0 + p] + 1, :])

                def pl(t, q):
                    return t[:, q * PLANE:(q + 1) * PLANE]

                ot = iop.tile([GROUP, 4 * PLANE], f16, tag="ot")
                # T[a,c] = j1[a,0]*V[0,c] + j1[a,1]*V[1,c]   (ab index q = 2a+b)
                # O[a,d] = T[a,0]*j2[d,0] + T[a,1]*j2[d,1]
                for a in range(2):
                    t0 = tmpp.tile([GROUP, PLANE], f16, tag="t0")
                    t1 = tmpp.tile([GROUP, PLANE], f16, tag="t1")
                    p0 = tmpp.tile([GROUP, PLANE], f16, tag="p0")
                    for c, tt in ((0, t0), (1, t1)):
                        nc.vector.tensor_mul(tt[:], pl(j1, 2 * a + 0), pl(vt, 0 + c))
                        nc.vector.tensor_mul(p0[:], pl(j1, 2 * a + 1), pl(vt, 2 + c))
                        nc.vector.tensor_add(tt[:], tt[:], p0[:])
                    for d in range(2):
                        q = 2 * a + d
                        nc.vector.tensor_mul(pl(ot, q), t0[:], pl(j2, 2 * d + 0))
                        nc.vector.tensor_mul(p0[:], t1[:], pl(j2, 2 * d + 1))
                        nc.vector.tensor_add(pl(ot, q), pl(ot, q), p0[:])
                for q in range(4):
                    nc.gpsimd.dma_start(O[q, n0:n0 + GROUP, :],
                                        pl(ot, q))

    _split_excess_waits(nc, mybir)
    return nc


def kernel(V_m, jones, ant1, ant2):
    from concourse.bass_utils import run_bass_kernel_spmd

    V_m = np.asarray(V_m, dtype=np.float32)
    jones = np.asarray(jones, dtype=np.float32)
    a1 = np.asarray(ant1).astype(np.int64)
    a2 = np.asarray(ant2).astype(np.int64)

    key = (a1.tobytes(), a2.tobytes())
    if key not in _cache:
        _cache[key] = _build(a1, a2)
    nc = _cache[key]

    in_maps = []
    for k in range(N_CORES):
        t0 = k * T_LOC
        vk = np.ascontiguousarray(
            V_m[:, :, :, t0:t0 + T_LOC, :]).reshape(4, NBL, PLANE)
        jk = np.ascontiguousarray(
            jones[:, :, :, t0:t0 + T_LOC, :]).reshape(4, NANT, PLANE)
        in_maps.append({"V": vk, "J": jk})

    res = run_bass_kernel_spmd(nc, in_maps, list(range(N_CORES)))
    out = np.empty((NPOL, NPOL, NBL, NTIMES, NFREQS), dtype=np.float32)
    for k in range(N_CORES):
        t0 = k * T_LOC
        out[:, :, :, t0:t0 + T_LOC, :] = res.results[k]["O"].reshape(
            NPOL, NPOL, NBL, T_LOC, NFREQS)
    return out

